# revision 17
# baseline (speedup 1.0000x reference)
"""BitNet linear layer (b1.58-style) on 8 Trainium2 NeuronCores.

Computes: scale = 1e-4 + mean(|W|); q = clip(round(W/scale), -1, 1);
          out = scale * (x @ q.T)
for x [4, 2048, 2048] f32 and W [8192, 2048] f32.

Sharding: tensor-parallel over out_features. Each core gets the full x
(replicated) and a 1024-row shard of the ternary q; cores run fully
independently and the host concatenates the per-core [8192, 1024]
output slices along the feature axis.

The elementwise prep runs once on the host (it is ~0.1% of the FLOPs
and would otherwise be redundantly recomputed per core): the exact
global scale and ternary q (bit-identical rounding vs the reference),
the f32->bf16 casts, and the transposes into SBUF-ready layouts.
`scale` is folded into the bf16 x cast, which is free in accuracy
terms (a single bf16 rounding either way), so the device applies no
scale at all. Remaining error is the bf16 rounding of x plus the bf16
output store (~2.2e-3 measured).

The device is then a pure gap-free bf16 matmul at the PE roofline:
2048 matmuls of N=512 at the 216 ns issue floor ~= 443 us, everything
else hidden behind it. Startup schedule (the only nontrivial part —
the SBUF-write fabric and per-queue DMA rates bound how fast q and the
first x tiles can land):

  - q ships as raw fp8 (ternary is exact; 2 MiB instead of 4) in 2-ko
    slices, alternating between the gpsimd (SWDGE, ~200 GB/s) and sync
    (HWDGE, ~100 GB/s) queues, and is expanded fp8 -> bf16 in SBUF by
    the DVE and ACT engines alternately (~1.3 us/slice each, idle at
    that point). The matmul reads bf16: an fp8 moving operand would
    stream ~20% slower (259 vs 216 ns/MM measured).
  - m-tiles 0 and 1 are interleaved ko-major so each q slice is
    consumed twice per arrival (0.86 us/ko consumption vs ~0.65 us/ko
    feed) — without this the PE stalls ~8 us on q arrival. Their x
    tiles ride the gpsimd queue in 512-col chunks woven between the
    q slices in need order.
  - ~10 dummy matmuls on an uninitialized SBUF tile (output never
    read) run during the preamble to carry the PE through the HAM
    SHORT window, so real matmuls start at 2.4 GHz, not 1.2.
  - Steady-state x rides the scalar queue as 1 MiB two-m-tile pair
    DMAs (higher HWDGE efficiency than 512 KiB singles and half the
    per-tile semaphore checks).
  - Per m-tile: 16 k-steps of two 512-col accumulating matmuls into a
    psum bank pair (8 banks -> 4 m-tiles in flight), DVE drains psum
    -> bf16 out tile, stores on the sync queue in natural [M, N-shard]
    orientation. The final m-tile runs its two psum sweeps
    sequentially so the first drain+store overlaps the second sweep.
"""

import sys

sys.path.insert(0, "/opt/trn_rl_repo")

import numpy as np
import ml_dtypes

import concourse.bass as bass
import concourse.tile as tile
from concourse import bacc, mybir
from concourse.bass_utils import run_bass_kernel_spmd

F32 = mybir.dt.float32
BF16 = mybir.dt.bfloat16
FP8 = mybir.dt.float8e4
U32 = mybir.dt.uint32
BF16_NP = ml_dtypes.bfloat16
FP8_NP = ml_dtypes.float8_e4m3

NCORES = 8
M = 8192          # tokens (4*2048)
K = 2048          # in_features
N_FULL = 8192     # out_features
NS = N_FULL // NCORES  # 1024 per-core shard
P = 128
KO = K // P       # 16 k-tiles
MT = M // P       # 64 m-tiles
NPAIR = MT // 2   # x pair-DMA rows


def build_nc():
    nc = bacc.Bacc("TRN2", target_bir_lowering=False, debug=False,
                   num_devices=NCORES)
    # x rows pair*128+p hold [j, ko*128+m] -> scale*x[(2*pair+j)*128+m, ko*128+p]
    x_d = nc.dram_tensor("x", [M // 2, 2 * K], BF16, kind="ExternalInput")
    q_d = nc.dram_tensor("q", [P, KO * NS], FP8, kind="ExternalInput")
    o_d = nc.dram_tensor("out", [M, NS], BF16, kind="ExternalOutput")
    x_ap, q_ap, o_ap = x_d.ap(), q_d.ap(), o_d.ap()

    with tile.TileContext(nc) as tc:
        with (
            tc.tile_pool(name="qpool", bufs=1) as qpool,
            tc.tile_pool(name="qstage", bufs=4) as qstage,
            tc.tile_pool(name="xspool", bufs=4) as xspool,
            tc.tile_pool(name="xppool", bufs=3) as xppool,
            tc.tile_pool(name="opool", bufs=4) as opool,
            tc.tile_pool(name="psum_o", bufs=8, space="PSUM") as psum_o,
        ):
            tile_q = qpool.tile([P, KO * NS], BF16, name="q")
            warm = qpool.tile([P, 640], BF16, name="warm")

            # ---- PE warmup (HAM) --------------------------------------
            wz = warm[:].bitcast(U32)
            nc.vector.tensor_scalar(wz, wz, 0, None,
                                    mybir.AluOpType.bitwise_and)
            psW = psum_o.tile([P, 512], F32, name="psW", tag="ps")
            for _ in range(10):
                nc.tensor.matmul(psW[:], lhsT=warm[:, 0:P],
                                 rhs=warm[:, P:640], start=True, stop=True)

            # ---- q slice loads (DMA issue order sets queue order) -----
            qs_t = {}
            def q_dma(g, eng):
                qs = qstage.tile([P, 2 * NS], FP8, name=f"qs_{g}", tag="qs")
                eng.dma_start(qs[:], q_ap[:, 2 * g * NS:2 * (g + 1) * NS])
                qs_t[g] = qs

            def q_expand(g, eng):
                eng(tile_q[:, 2 * g * NS:2 * (g + 1) * NS], qs_t[g][:])

            def dve_exp(dst, src):
                nc.vector.tensor_scalar(dst, src, 1.0, None,
                                        mybir.AluOpType.mult)

            def act_exp(dst, src):
                nc.scalar.copy(dst, src)

            # x singles for m-tiles 0-3 (pair row mt//2, column half mt%2)
            def xs_dma(mt, chunks, eng):
                xt = xspool.tile([P, K], BF16, name=f"x_{mt}", tag="x")
                r0, c0 = (mt // 2) * P, (mt % 2) * K
                step = K // chunks
                for c in range(chunks):
                    eng.dma_start(
                        xt[:, c * step:(c + 1) * step],
                        x_ap[r0:r0 + P, c0 + c * step:c0 + (c + 1) * step])
                return xt

            q_dma(0, nc.gpsimd)
            for g in (1, 3, 5, 7):
                q_dma(g, nc.sync)
            xt2 = xs_dma(2, 1, nc.scalar)
            xt3 = xs_dma(3, 1, nc.scalar)
            q_expand(0, dve_exp)

            # x0/x1 chunks woven between gpsimd q slices in need order
            xt0 = xspool.tile([P, K], BF16, name="x_0", tag="x")
            xt1 = xspool.tile([P, K], BF16, name="x_1", tag="x")
            def x01_chunk(xt, mt, c):
                nc.gpsimd.dma_start(
                    xt[:, c * 512:(c + 1) * 512],
                    x_ap[0:P, mt * K + c * 512:mt * K + (c + 1) * 512])

            x01_chunk(xt0, 0, 0); x01_chunk(xt1, 1, 0)
            x01_chunk(xt0, 0, 1); x01_chunk(xt1, 1, 1)
            q_expand(1, act_exp)
            q_dma(2, nc.gpsimd)
            q_expand(2, dve_exp)
            x01_chunk(xt0, 0, 2); x01_chunk(xt1, 1, 2)
            q_expand(3, act_exp)
            q_dma(4, nc.gpsimd)
            q_expand(4, dve_exp)
            x01_chunk(xt0, 0, 3); x01_chunk(xt1, 1, 3)
            q_dma(6, nc.gpsimd)
            q_expand(6, dve_exp)
            q_expand(5, act_exp)
            q_expand(7, act_exp)

            # ---- main loop: out[m, n] = sum_k x[m,k] q[n,k] -----------
            def mm_pair(ps2, xt, base, ko):
                nc.tensor.matmul(
                    ps2[0][:], lhsT=xt[:, base + ko * P:base + (ko + 1) * P],
                    rhs=tile_q[:, ko * NS:ko * NS + 512],
                    start=(ko == 0), stop=(ko == KO - 1))
                nc.tensor.matmul(
                    ps2[1][:], lhsT=xt[:, base + ko * P:base + (ko + 1) * P],
                    rhs=tile_q[:, ko * NS + 512:(ko + 1) * NS],
                    start=(ko == 0), stop=(ko == KO - 1))

            def drain_store(mt, ps2):
                ot = opool.tile([P, NS], BF16, name=f"o_{mt}", tag="o")
                nc.vector.tensor_scalar(
                    ot[:, 0:512], ps2[0][:], 1.0, None, mybir.AluOpType.mult)
                nc.vector.tensor_scalar(
                    ot[:, 512:1024], ps2[1][:], 1.0, None,
                    mybir.AluOpType.mult)
                nc.sync.dma_start(o_ap[mt * P:(mt + 1) * P, :], ot[:])

            def ps_pair(mt):
                return (psum_o.tile([P, 512], F32, name=f"psA_{mt}", tag="ps"),
                        psum_o.tile([P, 512], F32, name=f"psB_{mt}", tag="ps"))

            # m-tiles 0/1 interleaved ko-major (see header)
            ps0, ps1 = ps_pair(0), ps_pair(1)
            for ko in range(KO):
                mm_pair(ps0, xt0, 0, ko)
                mm_pair(ps1, xt1, 0, ko)
            drain_store(0, ps0)
            drain_store(1, ps1)

            for mt, xt in ((2, xt2), (3, xt3)):
                ps = ps_pair(mt)
                for ko in range(KO):
                    mm_pair(ps, xt, 0, ko)
                drain_store(mt, ps)

            # steady state: 1 MiB pair DMAs, two ko-sweeps per pair
            for pair in range(2, NPAIR):
                xt = xppool.tile([P, 2 * K], BF16, name=f"xp_{pair}",
                                 tag="xp")
                nc.scalar.dma_start(xt[:], x_ap[pair * P:(pair + 1) * P, :])
                for j in range(2):
                    mt = 2 * pair + j
                    if mt < MT - 1:
                        ps = ps_pair(mt)
                        for ko in range(KO):
                            mm_pair(ps, xt, j * K, ko)
                        drain_store(mt, ps)
                    else:
                        # last m-tile: sequential psum sweeps so the
                        # first drain+store overlaps the second sweep
                        psA, psB = ps_pair(mt)
                        ot = opool.tile([P, NS], BF16, name=f"o_{mt}",
                                        tag="o")
                        for ko in range(KO):
                            nc.tensor.matmul(
                                psA[:],
                                lhsT=xt[:, j * K + ko * P:
                                        j * K + (ko + 1) * P],
                                rhs=tile_q[:, ko * NS:ko * NS + 512],
                                start=(ko == 0), stop=(ko == KO - 1))
                        nc.vector.tensor_scalar(
                            ot[:, 0:512], psA[:], 1.0, None,
                            mybir.AluOpType.mult)
                        nc.sync.dma_start(
                            o_ap[mt * P:(mt + 1) * P, 0:512], ot[:, 0:512])
                        for ko in range(KO):
                            nc.tensor.matmul(
                                psB[:],
                                lhsT=xt[:, j * K + ko * P:
                                        j * K + (ko + 1) * P],
                                rhs=tile_q[:, ko * NS + 512:(ko + 1) * NS],
                                start=(ko == 0), stop=(ko == KO - 1))
                        nc.vector.tensor_scalar(
                            ot[:, 512:1024], psB[:], 1.0, None,
                            mybir.AluOpType.mult)
                        nc.sync.dma_start(
                            o_ap[mt * P:(mt + 1) * P, 512:1024],
                            ot[:, 512:1024])

    nc.compile()
    return nc


_NC_CACHE = None


def get_nc():
    global _NC_CACHE
    if _NC_CACHE is None:
        _NC_CACHE = build_nc()
    return _NC_CACHE


def make_in_maps(x, weight):
    x2 = np.asarray(x, dtype=np.float32).reshape(M, K)
    w = np.asarray(weight, dtype=np.float32)

    # exact reference prep: scale from the full W, ternary q
    scale = np.float32(1e-4) + np.abs(w).mean(dtype=np.float32)
    q = np.clip(np.rint(w / scale), -1.0, 1.0).astype(np.float32)

    # xdev[pair*128+p, j*2048 + ko*128+m] = scale*x[(2*pair+j)*128+m, ko*128+p]
    xs = (x2 * scale).reshape(NPAIR, 2, P, KO, P)  # [pair, j, m, ko, p]
    xdev = np.ascontiguousarray(
        xs.transpose(0, 4, 1, 3, 2).reshape(M // 2, 2 * K).astype(BF16_NP))

    # qdev_c[p, ko*1024+n] = q[c*1024+n, ko*128+p]  (ternary: exact in fp8)
    q4 = q.reshape(NCORES, NS, KO, P).transpose(0, 3, 2, 1)  # [c, p, ko, n]
    qdev = np.ascontiguousarray(q4.reshape(NCORES, P, KO * NS).astype(FP8_NP))

    return [{"x": xdev, "q": qdev[c]} for c in range(NCORES)]


def kernel(x, weight):
    nc = get_nc()
    in_maps = make_in_maps(x, weight)
    try:
        res = run_bass_kernel_spmd(nc, in_maps, list(range(NCORES)))
    except Exception:
        # transient device errors have been observed on first touch; retry once
        res = run_bass_kernel_spmd(nc, in_maps, list(range(NCORES)))
    out = np.concatenate(
        [np.asarray(res.results[c]["out"]) for c in range(NCORES)], axis=1)
    return np.ascontiguousarray(out, dtype=np.float32).reshape(4, 2048, N_FULL)
